# revision 37
# baseline (speedup 1.0000x reference)
"""MeanShift retrieval-KNN loss kernel for 8 Trainium2 NeuronCores — v3.

Reference computation (B=4096, K=32768, DIM=512, TOPK=5):
    query  = l2norm(query_raw); target = l2norm(target_raw)
    qbank  = l2norm(queue); qbank[0:B] = target
    dist_t = 2 - 2 * target @ qbank.T ; dist_q = 2 - 2 * query @ qbank.T
    idx    = top5 smallest dist_t per row
    loss   = mean_b( sum_j dist_q[b, idx[b,j]] / 5 )

Sharding: queue K axis split across 8 cores (4096 rows each); core 0's
shard is target_raw (the reference overwrites bank rows 0:B, and raw
queue rows 0:B are never read).

v3 design (vs v2):
  * t/q are NOT normalized on device: per-row ranking of sim_t is
    invariant to |t_b|, and the host corrects the sim_q payload by
    |q_b| (and computes exact dots for index-style candidates).
    Device preproc for t/q is just scale-cast + XBAR transpose + fp8.
  * grid coefficient 1024 (not 2048) so that an integer part up to 2047
    plus a 2^-13-granular index payload is exactly representable in
    fp32 (24-bit significand).
  * per [128,1024] psum pair, two styles (host replicates the rule):
      A  (3 of 4): ph1 DR -> ACT snap(+M, in psum) -> PE ones(-M)
                   -> ph2 DR (payload = FRS*|q|/sqrt(512)*sim_q) -> max8
      C2 (1 of 4): ph1 DR -> ACT snap(+M, psum->sbuf) ->
                   DVE stt((x-M)+iota*2^-13) -> max8
    A carries sim_q in the fraction; C2 carries the bank column index
    (host computes the exact dot for C2 winners).
  * engine budget/core (hot): PE ~205us, ACT ~165us, DVE ~230us target.
"""

import numpy as np

B, K, DIM, TOPK = 4096, 32768, 512, 5
NCORES = 8
KSH = K // NCORES  # 4096 bank rows per core

P = 128
CH = 512                      # psum-bank chunk width
PAIR = 2 * CH                 # 1024-wide psum pair (2 banks)
MAGIC = float(3 * (2 ** 22))  # fp32 add of +MAGIC snaps to integer grid
SQD = float(np.sqrt(DIM))     # 22.627
GRID = 1024.0                 # int part = round(GRID * (|t_b|/sqrt(D)) * sim_t)
TSC = 448.0                   # t scale: t_fp8 = t_raw * TSC/sqrt(D)
BSC = GRID / TSC              # bank scale (on normalized rows) = 2.2857
QSC = 0.5                     # q scale: q_fp8 = q_raw * QSC/sqrt(D)
FRS = QSC * BSC               # payload = FRS * (|q_b|/sqrt(D)) * sim_q
IDS = float(2.0 ** -13)       # index payload step
NCAND = 32                    # candidates per row per core (4 pairs x 8)

NB = B // P                   # 32 batch tiles
NPR = KSH // PAIR             # 4 pairs per batch tile
DCH = DIM // P                # 4 transpose chunks / 2 DR slice-pairs


def style_A(bt, pr):
    """True -> pair uses sim_q payload (ph2); False -> index payload.

    Exactly one C2 pair per batch tile, spread evenly in both pr-major
    (startup) and bt-major (steady) emission orders.
    """
    return pr != bt % 4


_CACHE = {}


def build_nc(b=B, ksh=KSH, dim=DIM, num_devices=NCORES):
    from contextlib import ExitStack

    import concourse.tile as tile
    from concourse import bacc, mybir
    from concourse.masks import make_identity

    f32 = mybir.dt.float32
    bf16 = mybir.dt.bfloat16
    fp8 = mybir.dt.float8e4
    Alu = mybir.AluOpType
    Act = mybir.ActivationFunctionType
    DR = mybir.MatmulPerfMode.DoubleRow

    NS = ksh // P             # 32 bank row-tiles
    TPP = NS // NPR           # 8 bank row-tiles per pair-column

    nc = bacc.Bacc(
        "TRN2", target_bir_lowering=False, debug=False, num_devices=num_devices
    )
    q_d = nc.dram_tensor("query_raw", [b, dim], f32, kind="ExternalInput").ap()
    t_d = nc.dram_tensor("target_raw", [b, dim], f32, kind="ExternalInput").ap()
    s_d = nc.dram_tensor("qshard", [ksh, dim], f32, kind="ExternalInput").ap()
    o_d = nc.dram_tensor("out", [b, NCAND], f32, kind="ExternalOutput").ap()

    MEG = 4                   # row-tiles per mega preproc group
    with tile.TileContext(nc) as tc, ExitStack() as ctx:
        singles = ctx.enter_context(tc.tile_pool(name="singles", bufs=1))
        ld = ctx.enter_context(tc.tile_pool(name="ld", bufs=12))
        nrm = ctx.enter_context(tc.tile_pool(name="nrm", bufs=6))
        small = ctx.enter_context(tc.tile_pool(name="small", bufs=8))
        psum = ctx.enter_context(tc.tile_pool(name="psum", bufs=3, space="PSUM"))
        pstp = ctx.enter_context(tc.tile_pool(name="pstp", bufs=2, space="PSUM"))
        c2p = ctx.enter_context(tc.tile_pool(name="c2p", bufs=3))

        identb = singles.tile([P, P], bf16)
        make_identity(nc, identb)
        onesc = singles.tile([1, P], bf16)
        mrow_n = singles.tile([1, CH], bf16)
        iota_raw = singles.tile([P, PAIR], f32)
        iotas = [singles.tile([P, PAIR], f32, name=f"iota{pr}")
                 for pr in range(NPR)]

        def emit_constants():
            nc.gpsimd.memset(onesc, 1.0)
            nc.gpsimd.memset(mrow_n, -MAGIC)
            # index payload tiles: iota_pr[j] = (pr*1024 + j) * 2^-13
            nc.gpsimd.iota(iota_raw, [[1, PAIR]], channel_multiplier=0,
                           allow_small_or_imprecise_dtypes=True)
            for pr in range(NPR):
                nc.vector.tensor_scalar(out=iotas[pr], in0=iota_raw,
                                        scalar1=IDS,
                                        scalar2=float(pr * PAIR) * IDS,
                                        op0=Alu.mult, op1=Alu.add)

        # Resident fp8 operands, DIM on partitions (DR slice layout).
        qbT = singles.tile([P, DCH, ksh], fp8)  # bank^T (normalized * BSC)
        tT = singles.tile([P, DCH, b], fp8)     # t_raw^T * TSC/sqrt(D)
        qT = singles.tile([P, DCH, b], fp8)     # q_raw^T * QSC/sqrt(D)

        cands = [singles.tile([P, NCAND], f32, name=f"cand{bt}")
                 for bt in range(NB)]

        def finish_tile(xn, dest, it, pfx, trans):
            """bf16 [P,dim] -> transpose -> fp8 resident slice."""
            dslc = dest[:, :, it * P:(it + 1) * P]
            if trans == 'pe':
                pt = pstp.tile([P, DCH, P], bf16, tag="pt", name=f"{pfx}pt{it}")
                for dc in range(DCH):
                    nc.tensor.transpose(pt[:, dc, :],
                                        xn[:, dc * P:(dc + 1) * P], identb)
                if it % 2 == 0:
                    nc.scalar.copy(dslc, pt)
                else:
                    nc.vector.tensor_copy(dslc, pt)
                return
            xt = nrm.tile([P, DCH, P], bf16, tag="xt", name=f"{pfx}xt{it}")
            nc.sync.dma_start_transpose(xt, xn)
            nc.gpsimd.dma_start(out=dslc, in_=xt)  # SWDGE fp8 cast

        def load_raw(x_dram, it, pfx):
            raw = ld.tile([P, dim], f32, tag="raw", name=f"{pfx}r{it}")
            nc.gpsimd.dma_start(out=raw, in_=x_dram[it * P:(it + 1) * P, :])
            return raw

        def tq_tile(it, trans='xbar'):
            """Scale-cast t (ACT) and q (DVE), then transpose+fp8."""
            raw_t = load_raw(t_d, it, "t")
            raw_q = load_raw(q_d, it, "q")
            xnt = nrm.tile([P, dim], bf16, tag="xnt", name=f"txn{it}")
            nc.scalar.activation(xnt, raw_t, Act.Copy, scale=TSC / SQD)
            finish_tile(xnt, tT, it, "t", trans)
            xnq = nrm.tile([P, dim], bf16, tag="xnq", name=f"qxn{it}")
            nc.vector.tensor_scalar(out=xnq, in0=raw_q, scalar1=QSC / SQD,
                                    scalar2=None, op0=Alu.mult)
            finish_tile(xnq, qT, it, "q", trans)

        def bank_tile(j, trans='xbar'):
            """l2norm * BSC via ACT square-accum + DVE rsqrt + DVE cast."""
            raw = load_raw(s_d, j, "s")
            ss = small.tile([P, 1], f32, tag="ss", name=f"sss{j}")
            sq = nrm.tile([P, dim], f32, tag="sq", name=f"ssq{j}")
            nc.scalar.activation(sq, raw, Act.Square, accum_out=ss)
            stdv = small.tile([P, 1], f32, tag="std", name=f"ssd{j}")
            nc.scalar.activation(stdv, ss, Act.Sqrt, scale=1.0 / (BSC * BSC))
            rin = small.tile([P, 1], f32, tag="rin", name=f"sri{j}")
            nc.vector.reciprocal(rin, stdv)
            xn = nrm.tile([P, dim], bf16, tag="xn", name=f"sxn{j}")
            nc.vector.tensor_scalar(out=xn, in0=raw, scalar1=rin,
                                    scalar2=None, op0=Alu.mult)
            finish_tile(xn, qbT, j, "s", trans)

        def ph1(bt, pr, close_group=False):
            bs = slice(bt * P, (bt + 1) * P)
            pv = psum.tile([P, PAIR], f32, tag="pv", name=f"pv{bt}_{pr}")
            for c in range(2):
                ks = slice((pr * 2 + c) * CH, (pr * 2 + c + 1) * CH)
                for dr in range(2):
                    nc.tensor.matmul(pv[:, c * CH:(c + 1) * CH],
                                     tT[:, 2 * dr:2 * dr + 2, bs],
                                     qbT[:, 2 * dr:2 * dr + 2, ks],
                                     start=(dr == 0),
                                     stop=(close_group and dr == 1),
                                     perf_mode=DR)
            return pv

        def snap_A(pv):
            nc.scalar.activation(pv, pv, Act.Copy, bias=MAGIC)

        def finish_A(bt, pr, pv):
            """ones(-M) + ph2 payload matmuls + max8 from psum."""
            bs = slice(bt * P, (bt + 1) * P)
            for c in range(2):
                nc.tensor.matmul(pv[:, c * CH:(c + 1) * CH], onesc, mrow_n,
                                 start=False, stop=False, skip_group_check=True)
            for c in range(2):
                ks = slice((pr * 2 + c) * CH, (pr * 2 + c + 1) * CH)
                for dr in range(2):
                    nc.tensor.matmul(pv[:, c * CH:(c + 1) * CH],
                                     qT[:, 2 * dr:2 * dr + 2, bs],
                                     qbT[:, 2 * dr:2 * dr + 2, ks],
                                     start=False, stop=(dr == 1), perf_mode=DR)
            nc.vector.max(cands[bt][:, pr * 8:(pr + 1) * 8], pv)

        def finish_C2(bt, pr, pv):
            """snap-evac to sbuf, -M + iota on DVE, max8 from sbuf."""
            ev = c2p.tile([P, PAIR], f32, tag="ev", name=f"ev{bt}_{pr}")
            nc.scalar.activation(ev, pv, Act.Copy, bias=MAGIC)
            nc.vector.scalar_tensor_tensor(out=ev, in0=ev, scalar=-MAGIC,
                                           in1=iotas[pr], op0=Alu.add,
                                           op1=Alu.add)
            nc.vector.max(cands[bt][:, pr * 8:(pr + 1) * 8], ev)

        # ---------------- emission ----------------
        # Startup is slice-major over the first SB batch tiles; per-tile
        # preproc is emitted inline (fully before any consumer), with the
        # first tiles going through PE transposes so the PE has work while
        # DMA streams.  Steady state is bt-major with PF-tile lookahead.
        PF = 4
        SB = min(12, NB)
        items = [(bt, pr) for pr in range(NPR) for bt in range(SB)] + \
                [(bt, pr) for bt in range(SB, NB) for pr in range(NPR)]
        pend = None  # (bt, pr, pv) awaiting finish_A
        for i, (bt, pr) in enumerate(items):
            startup = i < SB * NPR
            if i == 0:
                emit_constants()
                tq_tile(0, trans='pe')
                tq_tile(1, trans='pe')
                for j in range(TPP):
                    bank_tile(j, trans='pe')
            if startup:
                if pr == 0 and bt + 2 < SB:
                    tq_tile(bt + 2, trans=('pe' if bt < 2 else 'xbar'))
                if pr + 1 < NPR and bt < TPP:
                    bank_tile((pr + 1) * TPP + bt, trans='pe')
                if pr == NPR - 1 and bt % 3 == 0 and SB + bt // 3 < SB + PF:
                    tq_tile(SB + bt // 3)
            elif pr == 0 and PF <= bt < NB - PF:
                tq_tile(bt + PF)

            def fire_A(pbt, ppr, ppv):
                finish_A(pbt, ppr, ppv)
                if ppr == NPR - 1:
                    nc.gpsimd.dma_start(out=o_d[pbt * P:(pbt + 1) * P, :],
                                        in_=cands[pbt])

            isA = style_A(bt, pr)
            pv = ph1(bt, pr, close_group=not isA)
            if isA:
                snap_A(pv)
                if pend is not None:
                    fire_A(*pend)
                pend = (bt, pr, pv)
            else:
                if pend is not None:
                    fire_A(*pend)
                    pend = None
                finish_C2(bt, pr, pv)
                if pr == NPR - 1:
                    nc.gpsimd.dma_start(out=o_d[bt * P:(bt + 1) * P, :],
                                        in_=cands[bt])
        if pend is not None:
            fire_A(*pend)

    nc.compile()
    return nc


def _get_nc():
    key = (B, KSH, DIM, NCORES)
    if key not in _CACHE:
        _CACHE[key] = build_nc()
    return _CACHE[key]


def merge_host(cand_v, query_raw, target_raw, queue, topk=TOPK):
    """cand_v: [ncores, b, NCAND] packed values -> scalar loss.

    Per candidate slot s of core c for row r:
      pr = s // 8, bt = r // 128.
      v = int + frac, int = round(GRID * rho_t * sim_t).
      style A : frac = FRS * (|q_r|/sqrt(D)) * sim_q   (frac in (-.5,.5))
      style C2: frac = (pr*1024 + j) * 2^-13, bank col = c*KSH + pr*1024+j
    """
    nc_, b, ncand = cand_v.shape
    q = np.asarray(query_raw, dtype=np.float64)
    t = np.asarray(target_raw, dtype=np.float64)
    qu = np.asarray(queue, dtype=np.float64)
    qn = np.linalg.norm(q, axis=1)                      # |q_r|

    v = np.transpose(cand_v.astype(np.float64), (1, 0, 2))  # [b, nc, NCAND]
    v = v.reshape(b, nc_ * ncand)
    vint = np.round(v)
    frac = v - vint

    # style mask per (row, flat candidate slot): A iff pr != bt % 4
    slot = np.arange(nc_ * ncand) % ncand
    pr_of_slot = slot // 8                               # [nc*NCAND]
    bt_of_row = (np.arange(b) // P)                      # [b]
    isA = pr_of_slot[None, :] != (bt_of_row[:, None] % 4)

    # top-5 by packed value (ranking == int ranking up to grid ties)
    top_idx = np.argpartition(-v, topk - 1, axis=1)[:, :topk]   # [b, 5]
    rows = np.arange(b)[:, None]
    w_frac = frac[rows, top_idx]
    w_isA = isA[rows, top_idx]
    w_core = (top_idx // ncand)
    w_pr = (top_idx % ncand) // 8

    # style A: sim_q from payload
    sim_q = np.zeros((b, topk))
    coefA = FRS * (qn / SQD)                             # [b]
    sim_q = np.where(w_isA, w_frac / coefA[:, None], 0.0)

    # style C2: exact dot for winners
    c2_rows, c2_cols = np.nonzero(~w_isA)
    if c2_rows.size:
        j_local = np.rint(w_frac[c2_rows, c2_cols] / IDS).astype(np.int64)
        g = w_core[c2_rows, c2_cols] * KSH + j_local     # global bank row
        bank_rows = np.where((g < B)[:, None],
                             t[np.minimum(g, B - 1)],
                             qu[np.minimum(g, K - 1)])
        bank_rows = bank_rows / np.linalg.norm(bank_rows, axis=1, keepdims=True)
        qrows = q[c2_rows] / qn[c2_rows][:, None]
        sim_q[c2_rows, c2_cols] = np.einsum('ij,ij->i', qrows, bank_rows)

    dist_q = 2.0 - 2.0 * sim_q
    return np.float32(dist_q.mean())


def run_device(query_raw, target_raw, queue, **spmd_kwargs):
    from concourse.bass_utils import run_bass_kernel_spmd

    q = np.ascontiguousarray(np.asarray(query_raw, dtype=np.float32))
    t = np.ascontiguousarray(np.asarray(target_raw, dtype=np.float32))
    qu = np.ascontiguousarray(np.asarray(queue, dtype=np.float32))

    nc = _get_nc()
    in_maps = []
    for c in range(NCORES):
        shard = t if c == 0 else qu[c * KSH:(c + 1) * KSH]
        in_maps.append(
            {"query_raw": q, "target_raw": t,
             "qshard": np.ascontiguousarray(shard)}
        )
    bres = run_bass_kernel_spmd(nc, in_maps, list(range(NCORES)), **spmd_kwargs)
    cand = np.stack([bres.results[c]["out"] for c in range(NCORES)], axis=0)
    return merge_host(cand, q, t, qu), bres


def kernel(query_raw, target_raw, queue):
    loss, _ = run_device(query_raw, target_raw, queue)
    return loss


# revision 38
# speedup vs baseline: 1.1721x; 1.1721x over previous
"""MeanShift retrieval-KNN loss kernel for 8 Trainium2 NeuronCores — v3.

Reference computation (B=4096, K=32768, DIM=512, TOPK=5):
    query  = l2norm(query_raw); target = l2norm(target_raw)
    qbank  = l2norm(queue); qbank[0:B] = target
    dist_t = 2 - 2 * target @ qbank.T ; dist_q = 2 - 2 * query @ qbank.T
    idx    = top5 smallest dist_t per row
    loss   = mean_b( sum_j dist_q[b, idx[b,j]] / 5 )

Sharding: queue K axis split across 8 cores (4096 rows each); core 0's
shard is target_raw (the reference overwrites bank rows 0:B, and raw
queue rows 0:B are never read).

v3 design (vs v2):
  * t/q are NOT normalized on device: per-row ranking of sim_t is
    invariant to |t_b|, and the host corrects the sim_q payload by
    |q_b| (and computes exact dots for index-style candidates).
    Device preproc for t/q is just scale-cast + XBAR transpose + fp8.
  * grid coefficient 1024 (not 2048) so that an integer part up to 2047
    plus a 2^-13-granular index payload is exactly representable in
    fp32 (24-bit significand).
  * per [128,1024] psum pair, two styles (host replicates the rule):
      A  (3 of 4): ph1 DR -> ACT snap(+M, in psum) -> PE ones(-M)
                   -> ph2 DR (payload = FRS*|q|/sqrt(512)*sim_q) -> max8
      C2 (1 of 4): ph1 DR -> ACT snap(+M, psum->sbuf) ->
                   DVE stt((x-M)+iota*2^-13) -> max8
    A carries sim_q in the fraction; C2 carries the bank column index
    (host computes the exact dot for C2 winners).
  * engine budget/core (hot): PE ~205us, ACT ~165us, DVE ~230us target.
"""

import numpy as np

B, K, DIM, TOPK = 4096, 32768, 512, 5
NCORES = 8
KSH = K // NCORES  # 4096 bank rows per core

P = 128
CH = 512                      # psum-bank chunk width
PAIR = 2 * CH                 # 1024-wide psum pair (2 banks)
MAGIC = float(3 * (2 ** 22))  # fp32 add of +MAGIC snaps to integer grid
SQD = float(np.sqrt(DIM))     # 22.627
GRID = 1024.0                 # int part = round(GRID * (|t_b|/sqrt(D)) * sim_t)
TSC = 448.0                   # t scale: t_fp8 = t_raw * TSC/sqrt(D)
BSC = GRID / TSC              # bank scale (on normalized rows) = 2.2857
QSC = 0.5                     # q scale: q_fp8 = q_raw * QSC/sqrt(D)
FRS = QSC * BSC               # payload = FRS * (|q_b|/sqrt(D)) * sim_q
IDS = float(2.0 ** -13)       # index payload step
NCAND = 32                    # candidates per row per core (4 pairs x 8)

NB = B // P                   # 32 batch tiles
NPR = KSH // PAIR             # 4 pairs per batch tile
DCH = DIM // P                # 4 transpose chunks / 2 DR slice-pairs


def style_A(bt, pr):
    """True -> pair uses sim_q payload (ph2); False -> index payload.

    Exactly one C2 pair per batch tile, spread evenly in both pr-major
    (startup) and bt-major (steady) emission orders.
    """
    return pr != bt % 4


_CACHE = {}


def build_nc(b=B, ksh=KSH, dim=DIM, num_devices=NCORES):
    from contextlib import ExitStack

    import concourse.tile as tile
    from concourse import bacc, mybir
    from concourse.masks import make_identity

    f32 = mybir.dt.float32
    bf16 = mybir.dt.bfloat16
    fp8 = mybir.dt.float8e4
    Alu = mybir.AluOpType
    Act = mybir.ActivationFunctionType
    DR = mybir.MatmulPerfMode.DoubleRow

    NS = ksh // P             # 32 bank row-tiles
    TPP = NS // NPR           # 8 bank row-tiles per pair-column

    nc = bacc.Bacc(
        "TRN2", target_bir_lowering=False, debug=False, num_devices=num_devices
    )
    q_d = nc.dram_tensor("query_raw", [b, dim], f32, kind="ExternalInput").ap()
    t_d = nc.dram_tensor("target_raw", [b, dim], f32, kind="ExternalInput").ap()
    s_d = nc.dram_tensor("qshard", [ksh, dim], f32, kind="ExternalInput").ap()
    o_d = nc.dram_tensor("out", [b, NCAND], f32, kind="ExternalOutput").ap()

    MEG = 4                   # row-tiles per mega preproc group
    with tile.TileContext(nc) as tc, ExitStack() as ctx:
        singles = ctx.enter_context(tc.tile_pool(name="singles", bufs=1))
        ld = ctx.enter_context(tc.tile_pool(name="ld", bufs=12))
        nrm = ctx.enter_context(tc.tile_pool(name="nrm", bufs=6))
        small = ctx.enter_context(tc.tile_pool(name="small", bufs=8))
        psum = ctx.enter_context(tc.tile_pool(name="psum", bufs=3, space="PSUM"))
        pstp = ctx.enter_context(tc.tile_pool(name="pstp", bufs=2, space="PSUM"))
        c2p = ctx.enter_context(tc.tile_pool(name="c2p", bufs=3))

        identb = singles.tile([P, P], bf16)
        make_identity(nc, identb)
        onesc = singles.tile([1, P], bf16)
        mrow_n = singles.tile([1, CH], bf16)
        iota_raw = singles.tile([P, PAIR], f32)
        iotas = [singles.tile([P, PAIR], f32, name=f"iota{pr}")
                 for pr in range(NPR)]

        def emit_constants():
            nc.gpsimd.memset(onesc, 1.0)
            nc.gpsimd.memset(mrow_n, -MAGIC)
            # index payload tiles: iota_pr[j] = (pr*1024 + j) * 2^-13
            nc.gpsimd.iota(iota_raw, [[1, PAIR]], channel_multiplier=0,
                           allow_small_or_imprecise_dtypes=True)
            for pr in range(NPR):
                nc.vector.tensor_scalar(out=iotas[pr], in0=iota_raw,
                                        scalar1=IDS,
                                        scalar2=float(pr * PAIR) * IDS,
                                        op0=Alu.mult, op1=Alu.add)

        # Resident fp8 operands, DIM on partitions (DR slice layout).
        qbT = singles.tile([P, DCH, ksh], fp8)  # bank^T (normalized * BSC)
        tT = singles.tile([P, DCH, b], fp8)     # t_raw^T * TSC/sqrt(D)
        qT = singles.tile([P, DCH, b], fp8)     # q_raw^T * QSC/sqrt(D)

        cands = [singles.tile([P, NCAND], f32, name=f"cand{bt}")
                 for bt in range(NB)]

        def finish_tile(xn, dest, it, pfx, trans):
            """bf16 [P,dim] -> transpose -> fp8 resident slice."""
            dslc = dest[:, :, it * P:(it + 1) * P]
            if trans == 'pe':
                pt = pstp.tile([P, DCH, P], bf16, tag="pt", name=f"{pfx}pt{it}")
                for dc in range(DCH):
                    nc.tensor.transpose(pt[:, dc, :],
                                        xn[:, dc * P:(dc + 1) * P], identb)
                if it % 2 == 0:
                    nc.scalar.copy(dslc, pt)
                else:
                    nc.vector.tensor_copy(dslc, pt)
                return
            xt = nrm.tile([P, DCH, P], bf16, tag="xt", name=f"{pfx}xt{it}")
            nc.sync.dma_start_transpose(xt, xn)
            nc.gpsimd.dma_start(out=dslc, in_=xt)  # SWDGE fp8 cast

        def load_raw(x_dram, it, pfx):
            raw = ld.tile([P, dim], f32, tag="raw", name=f"{pfx}r{it}")
            nc.sync.dma_start(out=raw, in_=x_dram[it * P:(it + 1) * P, :])
            return raw

        def tq_tile(it, trans='xbar'):
            """Scale-cast t (ACT) and q (DVE), then transpose+fp8."""
            raw_t = load_raw(t_d, it, "t")
            raw_q = load_raw(q_d, it, "q")
            xnt = nrm.tile([P, dim], bf16, tag="xnt", name=f"txn{it}")
            nc.scalar.activation(xnt, raw_t, Act.Copy, scale=TSC / SQD)
            finish_tile(xnt, tT, it, "t", trans)
            xnq = nrm.tile([P, dim], bf16, tag="xnq", name=f"qxn{it}")
            nc.vector.tensor_scalar(out=xnq, in0=raw_q, scalar1=QSC / SQD,
                                    scalar2=None, op0=Alu.mult)
            finish_tile(xnq, qT, it, "q", trans)

        def bank_tile(j, trans='xbar'):
            """l2norm * BSC via ACT square-accum + DVE rsqrt + DVE cast."""
            raw = load_raw(s_d, j, "s")
            ss = small.tile([P, 1], f32, tag="ss", name=f"sss{j}")
            sq = nrm.tile([P, dim], f32, tag="sq", name=f"ssq{j}")
            nc.scalar.activation(sq, raw, Act.Square, accum_out=ss)
            stdv = small.tile([P, 1], f32, tag="std", name=f"ssd{j}")
            nc.scalar.activation(stdv, ss, Act.Sqrt, scale=1.0 / (BSC * BSC))
            rin = small.tile([P, 1], f32, tag="rin", name=f"sri{j}")
            nc.vector.reciprocal(rin, stdv)
            xn = nrm.tile([P, dim], bf16, tag="xn", name=f"sxn{j}")
            nc.vector.tensor_scalar(out=xn, in0=raw, scalar1=rin,
                                    scalar2=None, op0=Alu.mult)
            finish_tile(xn, qbT, j, "s", trans)

        def ph1(bt, pr, close_group=False):
            bs = slice(bt * P, (bt + 1) * P)
            pv = psum.tile([P, PAIR], f32, tag="pv", name=f"pv{bt}_{pr}")
            for c in range(2):
                ks = slice((pr * 2 + c) * CH, (pr * 2 + c + 1) * CH)
                for dr in range(2):
                    nc.tensor.matmul(pv[:, c * CH:(c + 1) * CH],
                                     tT[:, 2 * dr:2 * dr + 2, bs],
                                     qbT[:, 2 * dr:2 * dr + 2, ks],
                                     start=(dr == 0),
                                     stop=(close_group and dr == 1),
                                     perf_mode=DR)
            return pv

        def snap_A(pv):
            nc.scalar.activation(pv, pv, Act.Copy, bias=MAGIC)

        def finish_A(bt, pr, pv):
            """ones(-M) + ph2 payload matmuls + max8 from psum."""
            bs = slice(bt * P, (bt + 1) * P)
            for c in range(2):
                nc.tensor.matmul(pv[:, c * CH:(c + 1) * CH], onesc, mrow_n,
                                 start=False, stop=False, skip_group_check=True)
            for c in range(2):
                ks = slice((pr * 2 + c) * CH, (pr * 2 + c + 1) * CH)
                for dr in range(2):
                    nc.tensor.matmul(pv[:, c * CH:(c + 1) * CH],
                                     qT[:, 2 * dr:2 * dr + 2, bs],
                                     qbT[:, 2 * dr:2 * dr + 2, ks],
                                     start=False, stop=(dr == 1), perf_mode=DR)
            nc.vector.max(cands[bt][:, pr * 8:(pr + 1) * 8], pv)

        def finish_C2(bt, pr, pv):
            """snap-evac to sbuf, -M + iota on DVE, max8 from sbuf."""
            ev = c2p.tile([P, PAIR], f32, tag="ev", name=f"ev{bt}_{pr}")
            nc.scalar.activation(ev, pv, Act.Copy, bias=MAGIC)
            nc.vector.scalar_tensor_tensor(out=ev, in0=ev, scalar=-MAGIC,
                                           in1=iotas[pr], op0=Alu.add,
                                           op1=Alu.add)
            nc.vector.max(cands[bt][:, pr * 8:(pr + 1) * 8], ev)

        # ---------------- emission ----------------
        # Startup is slice-major over the first SB batch tiles; per-tile
        # preproc is emitted inline (fully before any consumer), with the
        # first tiles going through PE transposes so the PE has work while
        # DMA streams.  Steady state is bt-major with PF-tile lookahead.
        PF = 3
        SB = min(12, NB)
        items = [(bt, pr) for pr in range(NPR) for bt in range(SB)] + \
                [(bt, pr) for bt in range(SB, NB) for pr in range(NPR)]
        pend = None  # (bt, pr, pv) awaiting finish_A
        for i, (bt, pr) in enumerate(items):
            startup = i < SB * NPR
            if i == 0:
                emit_constants()
                tq_tile(0, trans='pe')
                tq_tile(1, trans='pe')
                for j in range(TPP):
                    bank_tile(j, trans='pe')
            if startup:
                if pr == 0 and bt + 2 < SB:
                    tq_tile(bt + 2, trans=('pe' if bt < 2 else 'xbar'))
                if pr + 1 < NPR and bt < TPP:
                    bank_tile((pr + 1) * TPP + bt, trans='pe')
                if pr == NPR - 1 and bt % 4 == 0 and SB + bt // 4 < SB + PF:
                    tq_tile(SB + bt // 4)
            elif pr == 0 and PF <= bt < NB - PF:
                tq_tile(bt + PF)

            isA = style_A(bt, pr)
            pv = ph1(bt, pr, close_group=not isA)
            if isA:
                snap_A(pv)
                if pend is not None:
                    finish_A(*pend)
                pend = (bt, pr, pv)
            else:
                if pend is not None:
                    finish_A(*pend)
                    pend = None
                finish_C2(bt, pr, pv)
        if pend is not None:
            finish_A(*pend)
        for bt in range(NB):
            nc.gpsimd.dma_start(out=o_d[bt * P:(bt + 1) * P, :], in_=cands[bt])

    nc.compile()
    return nc


def _get_nc():
    key = (B, KSH, DIM, NCORES)
    if key not in _CACHE:
        _CACHE[key] = build_nc()
    return _CACHE[key]


def merge_host(cand_v, query_raw, target_raw, queue, topk=TOPK):
    """cand_v: [ncores, b, NCAND] packed values -> scalar loss.

    Per candidate slot s of core c for row r:
      pr = s // 8, bt = r // 128.
      v = int + frac, int = round(GRID * rho_t * sim_t).
      style A : frac = FRS * (|q_r|/sqrt(D)) * sim_q   (frac in (-.5,.5))
      style C2: frac = (pr*1024 + j) * 2^-13, bank col = c*KSH + pr*1024+j
    """
    nc_, b, ncand = cand_v.shape
    q = np.asarray(query_raw, dtype=np.float64)
    t = np.asarray(target_raw, dtype=np.float64)
    qu = np.asarray(queue, dtype=np.float64)
    qn = np.linalg.norm(q, axis=1)                      # |q_r|

    v = np.transpose(cand_v.astype(np.float64), (1, 0, 2))  # [b, nc, NCAND]
    v = v.reshape(b, nc_ * ncand)
    vint = np.round(v)
    frac = v - vint

    # style mask per (row, flat candidate slot): A iff pr != bt % 4
    slot = np.arange(nc_ * ncand) % ncand
    pr_of_slot = slot // 8                               # [nc*NCAND]
    bt_of_row = (np.arange(b) // P)                      # [b]
    isA = pr_of_slot[None, :] != (bt_of_row[:, None] % 4)

    # top-5 by packed value (ranking == int ranking up to grid ties)
    top_idx = np.argpartition(-v, topk - 1, axis=1)[:, :topk]   # [b, 5]
    rows = np.arange(b)[:, None]
    w_frac = frac[rows, top_idx]
    w_isA = isA[rows, top_idx]
    w_core = (top_idx // ncand)
    w_pr = (top_idx % ncand) // 8

    # style A: sim_q from payload
    sim_q = np.zeros((b, topk))
    coefA = FRS * (qn / SQD)                             # [b]
    sim_q = np.where(w_isA, w_frac / coefA[:, None], 0.0)

    # style C2: exact dot for winners
    c2_rows, c2_cols = np.nonzero(~w_isA)
    if c2_rows.size:
        j_local = np.rint(w_frac[c2_rows, c2_cols] / IDS).astype(np.int64)
        g = w_core[c2_rows, c2_cols] * KSH + j_local     # global bank row
        bank_rows = np.where((g < B)[:, None],
                             t[np.minimum(g, B - 1)],
                             qu[np.minimum(g, K - 1)])
        bank_rows = bank_rows / np.linalg.norm(bank_rows, axis=1, keepdims=True)
        qrows = q[c2_rows] / qn[c2_rows][:, None]
        sim_q[c2_rows, c2_cols] = np.einsum('ij,ij->i', qrows, bank_rows)

    dist_q = 2.0 - 2.0 * sim_q
    return np.float32(dist_q.mean())


def run_device(query_raw, target_raw, queue, **spmd_kwargs):
    from concourse.bass_utils import run_bass_kernel_spmd

    q = np.ascontiguousarray(np.asarray(query_raw, dtype=np.float32))
    t = np.ascontiguousarray(np.asarray(target_raw, dtype=np.float32))
    qu = np.ascontiguousarray(np.asarray(queue, dtype=np.float32))

    nc = _get_nc()
    in_maps = []
    for c in range(NCORES):
        shard = t if c == 0 else qu[c * KSH:(c + 1) * KSH]
        in_maps.append(
            {"query_raw": q, "target_raw": t,
             "qshard": np.ascontiguousarray(shard)}
        )
    bres = run_bass_kernel_spmd(nc, in_maps, list(range(NCORES)), **spmd_kwargs)
    cand = np.stack([bres.results[c]["out"] for c in range(NCORES)], axis=0)
    return merge_host(cand, q, t, qu), bres


def kernel(query_raw, target_raw, queue):
    loss, _ = run_device(query_raw, target_raw, queue)
    return loss


# revision 39
# speedup vs baseline: 1.2052x; 1.0283x over previous
"""MeanShift retrieval-KNN loss kernel for 8 Trainium2 NeuronCores — v3.

Reference computation (B=4096, K=32768, DIM=512, TOPK=5):
    query  = l2norm(query_raw); target = l2norm(target_raw)
    qbank  = l2norm(queue); qbank[0:B] = target
    dist_t = 2 - 2 * target @ qbank.T ; dist_q = 2 - 2 * query @ qbank.T
    idx    = top5 smallest dist_t per row
    loss   = mean_b( sum_j dist_q[b, idx[b,j]] / 5 )

Sharding: queue K axis split across 8 cores (4096 rows each); core 0's
shard is target_raw (the reference overwrites bank rows 0:B, and raw
queue rows 0:B are never read).

v3 design (vs v2):
  * t/q are NOT normalized on device: per-row ranking of sim_t is
    invariant to |t_b|, and the host corrects the sim_q payload by
    |q_b| (and computes exact dots for index-style candidates).
    Device preproc for t/q is just scale-cast + XBAR transpose + fp8.
  * grid coefficient 1024 (not 2048) so that an integer part up to 2047
    plus a 2^-13-granular index payload is exactly representable in
    fp32 (24-bit significand).
  * per [128,1024] psum pair, two styles (host replicates the rule):
      A  (3 of 4): ph1 DR -> ACT snap(+M, in psum) -> PE ones(-M)
                   -> ph2 DR (payload = FRS*|q|/sqrt(512)*sim_q) -> max8
      C2 (1 of 4): ph1 DR -> ACT snap(+M, psum->sbuf) ->
                   DVE stt((x-M)+iota*2^-13) -> max8
    A carries sim_q in the fraction; C2 carries the bank column index
    (host computes the exact dot for C2 winners).
  * engine budget/core (hot): PE ~205us, ACT ~165us, DVE ~230us target.
"""

import numpy as np

B, K, DIM, TOPK = 4096, 32768, 512, 5
NCORES = 8
KSH = K // NCORES  # 4096 bank rows per core

P = 128
CH = 512                      # psum-bank chunk width
PAIR = 2 * CH                 # 1024-wide psum pair (2 banks)
MAGIC = float(3 * (2 ** 22))  # fp32 add of +MAGIC snaps to integer grid
SQD = float(np.sqrt(DIM))     # 22.627
GRID = 1024.0                 # int part = round(GRID * (|t_b|/sqrt(D)) * sim_t)
TSC = 448.0                   # t scale: t_fp8 = t_raw * TSC/sqrt(D)
BSC = GRID / TSC              # bank scale (on normalized rows) = 2.2857
QSC = 0.5                     # q scale: q_fp8 = q_raw * QSC/sqrt(D)
FRS = QSC * BSC               # payload = FRS * (|q_b|/sqrt(D)) * sim_q
IDS = float(2.0 ** -13)       # index payload step
NCAND = 32                    # candidates per row per core (4 pairs x 8)

NB = B // P                   # 32 batch tiles
NPR = KSH // PAIR             # 4 pairs per batch tile
DCH = DIM // P                # 4 transpose chunks / 2 DR slice-pairs


def style_A(bt, pr):
    """True -> pair uses sim_q payload (ph2); False -> index payload.

    Exactly one C2 pair per batch tile, spread evenly in both pr-major
    (startup) and bt-major (steady) emission orders.
    """
    return pr != bt % 4


_CACHE = {}


def build_nc(b=B, ksh=KSH, dim=DIM, num_devices=NCORES):
    from contextlib import ExitStack

    import concourse.tile as tile
    from concourse import bacc, mybir
    from concourse.masks import make_identity

    f32 = mybir.dt.float32
    bf16 = mybir.dt.bfloat16
    fp8 = mybir.dt.float8e4
    Alu = mybir.AluOpType
    Act = mybir.ActivationFunctionType
    DR = mybir.MatmulPerfMode.DoubleRow

    NS = ksh // P             # 32 bank row-tiles
    TPP = NS // NPR           # 8 bank row-tiles per pair-column

    nc = bacc.Bacc(
        "TRN2", target_bir_lowering=False, debug=False, num_devices=num_devices
    )
    q_d = nc.dram_tensor("query_raw", [b, dim], f32, kind="ExternalInput").ap()
    t_d = nc.dram_tensor("target_raw", [b, dim], f32, kind="ExternalInput").ap()
    s_d = nc.dram_tensor("qshard", [ksh, dim], f32, kind="ExternalInput").ap()
    o_d = nc.dram_tensor("out", [b, NCAND], f32, kind="ExternalOutput").ap()

    MEG = 4                   # row-tiles per mega preproc group
    with tile.TileContext(nc) as tc, ExitStack() as ctx:
        singles = ctx.enter_context(tc.tile_pool(name="singles", bufs=1))
        ld = ctx.enter_context(tc.tile_pool(name="ld", bufs=12))
        nrm = ctx.enter_context(tc.tile_pool(name="nrm", bufs=10))
        small = ctx.enter_context(tc.tile_pool(name="small", bufs=8))
        psum = ctx.enter_context(tc.tile_pool(name="psum", bufs=3, space="PSUM"))
        pstp = ctx.enter_context(tc.tile_pool(name="pstp", bufs=2, space="PSUM"))
        c2p = ctx.enter_context(tc.tile_pool(name="c2p", bufs=3))

        identb = singles.tile([P, P], bf16)
        make_identity(nc, identb)
        onesc = singles.tile([1, P], bf16)
        mrow_n = singles.tile([1, CH], bf16)
        iota_raw = singles.tile([P, PAIR], f32)
        iotas = [singles.tile([P, PAIR], f32, name=f"iota{pr}")
                 for pr in range(NPR)]

        def emit_constants():
            nc.gpsimd.memset(onesc, 1.0)
            nc.gpsimd.memset(mrow_n, -MAGIC)
            # index payload tiles: iota_pr[j] = (pr*1024 + j) * 2^-13
            nc.gpsimd.iota(iota_raw, [[1, PAIR]], channel_multiplier=0,
                           allow_small_or_imprecise_dtypes=True)
            for pr in range(NPR):
                nc.vector.tensor_scalar(out=iotas[pr], in0=iota_raw,
                                        scalar1=IDS,
                                        scalar2=float(pr * PAIR) * IDS,
                                        op0=Alu.mult, op1=Alu.add)

        # Resident fp8 operands, DIM on partitions (DR slice layout).
        qbT = singles.tile([P, DCH, ksh], fp8)  # bank^T (normalized * BSC)
        tT = singles.tile([P, DCH, b], fp8)     # t_raw^T * TSC/sqrt(D)
        qT = singles.tile([P, DCH, b], fp8)     # q_raw^T * QSC/sqrt(D)

        cands = [singles.tile([P, NCAND], f32, name=f"cand{bt}")
                 for bt in range(NB)]

        def finish_tile(xn, dest, it, pfx, trans):
            """bf16 [P,dim] -> transpose -> fp8 resident slice."""
            dslc = dest[:, :, it * P:(it + 1) * P]
            if trans == 'pe':
                pt = pstp.tile([P, DCH, P], bf16, tag="pt", name=f"{pfx}pt{it}")
                for dc in range(DCH):
                    nc.tensor.transpose(pt[:, dc, :],
                                        xn[:, dc * P:(dc + 1) * P], identb)
                if it % 2 == 0:
                    nc.scalar.copy(dslc, pt)
                else:
                    nc.vector.tensor_copy(dslc, pt)
                return
            xt = nrm.tile([P, DCH, P], bf16, tag="xt", name=f"{pfx}xt{it}")
            nc.sync.dma_start_transpose(xt, xn)
            nc.gpsimd.dma_start(out=dslc, in_=xt)  # SWDGE fp8 cast

        def load_raw(x_dram, it, pfx):
            raw = ld.tile([P, dim], f32, tag="raw", name=f"{pfx}r{it}")
            nc.sync.dma_start(out=raw, in_=x_dram[it * P:(it + 1) * P, :])
            return raw

        def tq_tile(it, trans='xbar'):
            """Scale-cast t (ACT) and q (DVE), then transpose+fp8."""
            raw_t = load_raw(t_d, it, "t")
            raw_q = load_raw(q_d, it, "q")
            xnt = nrm.tile([P, dim], bf16, tag="xnt", name=f"txn{it}")
            nc.scalar.activation(xnt, raw_t, Act.Copy, scale=TSC / SQD)
            finish_tile(xnt, tT, it, "t", trans)
            xnq = nrm.tile([P, dim], bf16, tag="xnq", name=f"qxn{it}")
            nc.vector.tensor_scalar(out=xnq, in0=raw_q, scalar1=QSC / SQD,
                                    scalar2=None, op0=Alu.mult)
            finish_tile(xnq, qT, it, "q", trans)

        def bank_tile(j, trans='xbar'):
            """l2norm * BSC via ACT square-accum + DVE rsqrt + DVE cast."""
            raw = load_raw(s_d, j, "s")
            ss = small.tile([P, 1], f32, tag="ss", name=f"sss{j}")
            sq = nrm.tile([P, dim], f32, tag="sq", name=f"ssq{j}")
            nc.scalar.activation(sq, raw, Act.Square, accum_out=ss)
            stdv = small.tile([P, 1], f32, tag="std", name=f"ssd{j}")
            nc.scalar.activation(stdv, ss, Act.Sqrt, scale=1.0 / (BSC * BSC))
            rin = small.tile([P, 1], f32, tag="rin", name=f"sri{j}")
            nc.vector.reciprocal(rin, stdv)
            xn = nrm.tile([P, dim], bf16, tag="xn", name=f"sxn{j}")
            nc.vector.tensor_scalar(out=xn, in0=raw, scalar1=rin,
                                    scalar2=None, op0=Alu.mult)
            finish_tile(xn, qbT, j, "s", trans)

        def ph1(bt, pr, close_group=False):
            bs = slice(bt * P, (bt + 1) * P)
            pv = psum.tile([P, PAIR], f32, tag="pv", name=f"pv{bt}_{pr}")
            for c in range(2):
                ks = slice((pr * 2 + c) * CH, (pr * 2 + c + 1) * CH)
                for dr in range(2):
                    nc.tensor.matmul(pv[:, c * CH:(c + 1) * CH],
                                     tT[:, 2 * dr:2 * dr + 2, bs],
                                     qbT[:, 2 * dr:2 * dr + 2, ks],
                                     start=(dr == 0),
                                     stop=(close_group and dr == 1),
                                     perf_mode=DR)
            return pv

        def snap_A(pv):
            nc.scalar.activation(pv, pv, Act.Copy, bias=MAGIC)

        def finish_A(bt, pr, pv):
            """ones(-M) + ph2 payload matmuls + max8 from psum."""
            bs = slice(bt * P, (bt + 1) * P)
            for c in range(2):
                nc.tensor.matmul(pv[:, c * CH:(c + 1) * CH], onesc, mrow_n,
                                 start=False, stop=False, skip_group_check=True)
            for c in range(2):
                ks = slice((pr * 2 + c) * CH, (pr * 2 + c + 1) * CH)
                for dr in range(2):
                    nc.tensor.matmul(pv[:, c * CH:(c + 1) * CH],
                                     qT[:, 2 * dr:2 * dr + 2, bs],
                                     qbT[:, 2 * dr:2 * dr + 2, ks],
                                     start=False, stop=(dr == 1), perf_mode=DR)
            nc.vector.max(cands[bt][:, pr * 8:(pr + 1) * 8], pv)

        def finish_C2(bt, pr, pv):
            """snap-evac to sbuf, -M + iota on DVE, max8 from sbuf."""
            ev = c2p.tile([P, PAIR], f32, tag="ev", name=f"ev{bt}_{pr}")
            nc.scalar.activation(ev, pv, Act.Copy, bias=MAGIC)
            nc.vector.scalar_tensor_tensor(out=ev, in0=ev, scalar=-MAGIC,
                                           in1=iotas[pr], op0=Alu.add,
                                           op1=Alu.add)
            nc.vector.max(cands[bt][:, pr * 8:(pr + 1) * 8], ev)

        # ---------------- emission ----------------
        # Startup is slice-major over the first SB batch tiles; per-tile
        # preproc is emitted inline (fully before any consumer), with the
        # first tiles going through PE transposes so the PE has work while
        # DMA streams.  Steady state is bt-major with PF-tile lookahead.
        PF = 3
        SB = min(12, NB)
        items = [(bt, pr) for pr in range(NPR) for bt in range(SB)] + \
                [(bt, pr) for bt in range(SB, NB) for pr in range(NPR)]
        pend = None  # (bt, pr, pv) awaiting finish_A
        for i, (bt, pr) in enumerate(items):
            startup = i < SB * NPR
            if i == 0:
                emit_constants()
                tq_tile(0, trans='pe')
                tq_tile(1, trans='pe')
                for j in range(TPP):
                    bank_tile(j, trans='pe')
            if startup:
                if pr == 0 and bt + 2 < SB:
                    tq_tile(bt + 2, trans=('pe' if bt < 2 else 'xbar'))
                if pr + 1 < NPR and bt < TPP:
                    bank_tile((pr + 1) * TPP + bt, trans='pe')
                if pr == NPR - 1 and bt % 4 == 0 and SB + bt // 4 < SB + PF:
                    tq_tile(SB + bt // 4)
            elif pr == 0 and PF <= bt < NB - PF:
                tq_tile(bt + PF)

            isA = style_A(bt, pr)
            pv = ph1(bt, pr, close_group=not isA)
            if isA:
                snap_A(pv)
                if pend is not None:
                    finish_A(*pend)
                pend = (bt, pr, pv)
            else:
                if pend is not None:
                    finish_A(*pend)
                    pend = None
                finish_C2(bt, pr, pv)
        if pend is not None:
            finish_A(*pend)
        for bt in range(NB):
            nc.gpsimd.dma_start(out=o_d[bt * P:(bt + 1) * P, :], in_=cands[bt])

    nc.compile()
    return nc


def _get_nc():
    key = (B, KSH, DIM, NCORES)
    if key not in _CACHE:
        _CACHE[key] = build_nc()
    return _CACHE[key]


def merge_host(cand_v, query_raw, target_raw, queue, topk=TOPK):
    """cand_v: [ncores, b, NCAND] packed values -> scalar loss.

    Per candidate slot s of core c for row r:
      pr = s // 8, bt = r // 128.
      v = int + frac, int = round(GRID * rho_t * sim_t).
      style A : frac = FRS * (|q_r|/sqrt(D)) * sim_q   (frac in (-.5,.5))
      style C2: frac = (pr*1024 + j) * 2^-13, bank col = c*KSH + pr*1024+j
    """
    nc_, b, ncand = cand_v.shape
    q = np.asarray(query_raw, dtype=np.float64)
    t = np.asarray(target_raw, dtype=np.float64)
    qu = np.asarray(queue, dtype=np.float64)
    qn = np.linalg.norm(q, axis=1)                      # |q_r|

    v = np.transpose(cand_v.astype(np.float64), (1, 0, 2))  # [b, nc, NCAND]
    v = v.reshape(b, nc_ * ncand)
    v = np.nan_to_num(v, nan=-1e9, posinf=-1e9, neginf=-1e9)
    vint = np.round(v)
    frac = v - vint

    # style mask per (row, flat candidate slot): A iff pr != bt % 4
    slot = np.arange(nc_ * ncand) % ncand
    pr_of_slot = slot // 8                               # [nc*NCAND]
    bt_of_row = (np.arange(b) // P)                      # [b]
    isA = pr_of_slot[None, :] != (bt_of_row[:, None] % 4)

    # top-5 by packed value (ranking == int ranking up to grid ties)
    top_idx = np.argpartition(-v, topk - 1, axis=1)[:, :topk]   # [b, 5]
    rows = np.arange(b)[:, None]
    w_frac = frac[rows, top_idx]
    w_isA = isA[rows, top_idx]
    w_core = (top_idx // ncand)
    w_pr = (top_idx % ncand) // 8

    # style A: sim_q from payload
    sim_q = np.zeros((b, topk))
    coefA = FRS * (qn / SQD)                             # [b]
    sim_q = np.where(w_isA, w_frac / coefA[:, None], 0.0)

    # style C2: exact dot for winners
    c2_rows, c2_cols = np.nonzero(~w_isA)
    if c2_rows.size:
        j_local = np.rint(w_frac[c2_rows, c2_cols] / IDS)
        j_local = np.clip(np.nan_to_num(j_local), 0, KSH - 1).astype(np.int64)
        g = w_core[c2_rows, c2_cols] * KSH + j_local     # global bank row
        bank_rows = np.where((g < B)[:, None],
                             t[np.minimum(g, B - 1)],
                             qu[np.minimum(g, K - 1)])
        bank_rows = bank_rows / np.linalg.norm(bank_rows, axis=1, keepdims=True)
        qrows = q[c2_rows] / qn[c2_rows][:, None]
        sim_q[c2_rows, c2_cols] = np.einsum('ij,ij->i', qrows, bank_rows)

    dist_q = 2.0 - 2.0 * sim_q
    return np.float32(dist_q.mean())


def run_device(query_raw, target_raw, queue, **spmd_kwargs):
    from concourse.bass_utils import run_bass_kernel_spmd

    q = np.ascontiguousarray(np.asarray(query_raw, dtype=np.float32))
    t = np.ascontiguousarray(np.asarray(target_raw, dtype=np.float32))
    qu = np.ascontiguousarray(np.asarray(queue, dtype=np.float32))

    nc = _get_nc()
    in_maps = []
    for c in range(NCORES):
        shard = t if c == 0 else qu[c * KSH:(c + 1) * KSH]
        in_maps.append(
            {"query_raw": q, "target_raw": t,
             "qshard": np.ascontiguousarray(shard)}
        )
    bres = run_bass_kernel_spmd(nc, in_maps, list(range(NCORES)), **spmd_kwargs)
    cand = np.stack([bres.results[c]["out"] for c in range(NCORES)], axis=0)
    return merge_host(cand, q, t, qu), bres


def kernel(query_raw, target_raw, queue):
    loss, _ = run_device(query_raw, target_raw, queue)
    return loss
